# revision 24
# baseline (speedup 1.0000x reference)
"""Trainium2 Bass kernel for CellPathwayAttentionAggregator (segment-reduce).

Math: out[b, s] = sum_{i in set s} softmax_s(attn_logits)[i] * G[b, flat_idx[i]]

Device decomposition (per core, transposed output):
    out^T = (W_exp^T @ G^T) * (1 / denom)[:, None]
where W_exp[g, s] = sum_{i in set s, flat_idx[i]=g} exp(attn_logits[i]) is the
(unnormalized) sparse aggregation matrix, scattered on the host as pure layout
prep (elementwise exp + scatter; no reductions on host), and
    denom[s] = sum_{i in set s} exp(attn_logits[i])
is computed ON DEVICE from a sets-on-partitions padded logits tile (ACT exp ->
DVE free-axis reduce -> DVE reciprocal; no PE involvement), followed by an
on-device per-partition normalization of the matmul output. The host
transposes each core's (sets x batch) block during assembly.

Sharding: 8 cores = 2 batch groups (512 rows) x 4 set groups (512 sets).
Each core accumulates a (512 x 8192) @ (8192 x 512) bf16 matmul in fp32 PSUM
over 64 K-tiles (4 set-subtile PSUM banks, N=512 moving operand), with a
dependency-free PE warmup against the HAM clock-gate and input tiles streamed
as 256KB G^T|W DMAs alternating the two HWDGE rings (which share a ~285 GB/s
aggregate ceiling - the kernel is input-bandwidth-bound). The output is
normalized into bf16 (host upcasts) and stored as partition-split DMAs on
both rings; the padded-logits load is deferred to the stream's tail.
"""

import sys

if "/opt/trn_rl_repo" not in sys.path:
    sys.path.insert(0, "/opt/trn_rl_repo")

import ml_dtypes
import numpy as np

NUM_SETS = 2048
NUM_GENESETS = 8192
BATCH = 1024
N_CORES = 8
BG, SG = 2, 4  # batch groups x set groups (BG*SG == N_CORES)
B_C = BATCH // BG  # 512 batch rows per core
S_C = NUM_SETS // SG  # 512 sets per core
P = 128
K_TILES = NUM_GENESETS // P  # 64
M_TILES = B_C // P  # 4
PAD_SLOTS = 128  # >= MAX set size (120)
NEG_FILL = -87.0  # exp(-87) ~ 1.6e-38 ~ 0 in fp32

_PROGRAM_CACHE = {}
LAST_RESULTS = None  # BassKernelResults of the most recent run (for profiling)


def _build_program():
    import concourse.mybir as mybir
    from concourse import bacc
    from concourse.tile import TileContext

    f32 = mybir.dt.float32
    bf16 = mybir.dt.bfloat16

    nc = bacc.Bacc("TRN2", target_bir_lowering=False, debug=False)
    # fused per-K-tile input: [:, :, :B_C] = G^T tile, [:, :, B_C:] = W tile.
    # One DMA per K-tile keeps every matmul's sync-wait count at <=1 (the
    # S3 LDWEIGHTS encoding only has a single wait slot).
    gw_d = nc.dram_tensor("gw", [K_TILES, P, B_C + S_C], bf16, kind="ExternalInput")
    plog_d = nc.dram_tensor(
        "plog", [P, (S_C // P) * PAD_SLOTS], f32, kind="ExternalInput"
    )
    out_d = nc.dram_tensor("out", [S_C, B_C], f32, kind="ExternalOutput")

    with TileContext(nc) as tc:
        with (
            tc.tile_pool(name="const", bufs=1) as cpool,
            tc.tile_pool(name="gw", bufs=12) as gwpool,
            tc.tile_pool(name="outp", bufs=4) as opool,
            tc.tile_pool(name="ps", bufs=1, space="PSUM") as ppool,
        ):
            # --- PE warmup: dependency-free N=1 matmuls on the pre-barrier
            # const tile keep the HAM clock-gate busy from right after the
            # entry barrier, so it reaches 8/8 (2.4 GHz) before the real
            # stream starts.
            const_one = nc.const_aps.aps[(bf16, 1.0)]
            scratch_ps = ppool.tile([1, 1], f32, tag="scratch")
            for _ in range(64):
                nc.tensor.matmul(
                    scratch_ps[:], const_one, const_one, start=True, stop=True
                )

            # --- tile 0 split across BOTH HWDGE rings (G-half on SP, W-half
            # on ACT) so the first matmul's data lands ~1us sooner; emitted
            # before the exp so ACT's ring isn't blocked behind the plog wait
            gw0 = gwpool.tile([P, B_C + S_C], bf16, tag="gw", name="gw0")
            nc.sync.dma_start(out=gw0[:, 0:B_C], in_=gw_d[0, :, 0:B_C])
            nc.scalar.dma_start(
                out=gw0[:, B_C : B_C + S_C], in_=gw_d[0, :, B_C : B_C + S_C]
            )

            # --- denominator chain: sets live on the PARTITION axis, so it
            # needs no PE matmuls at all (ACT exp -> DVE free-axis reduce ->
            # DVE reciprocal), fully parallel to the matmul stream ---
            SUBT = S_C // P  # 4 set-subtiles of 128 sets
            plog_sb = cpool.tile([P, SUBT * PAD_SLOTS], f32, tag="plog")
            nc.gpsimd.dma_start(out=plog_sb[:], in_=plog_d[:, :])
            exp_sb = cpool.tile([P, SUBT * PAD_SLOTS], f32, tag="exp")
            nc.scalar.activation(
                exp_sb[:], plog_sb[:], mybir.ActivationFunctionType.Exp
            )
            den_sb = cpool.tile([P, SUBT], f32, tag="den")
            nc.vector.tensor_reduce(
                out=den_sb[:],
                in_=exp_sb[:].rearrange("p (j t) -> p j t", t=PAD_SLOTS),
                op=mybir.AluOpType.add,
                axis=mybir.AxisListType.X,
            )
            recip_sb = cpool.tile([P, SUBT], f32, tag="recip")
            nc.vector.reciprocal(recip_sb[:], den_sb[:])

            # --- main matmul: out^T = W_c^T @ G_c^T, accumulated over 64
            # K-tiles; output has sets on partitions, batch on free ---
            acc = [
                ppool.tile([P, B_C], f32, tag=f"acc{j}", name=f"acc{j}")
                for j in range(SUBT)
            ]
            for k in range(K_TILES):
                if k == 0:
                    gw_sb = gw0
                else:
                    gw_sb = gwpool.tile([P, B_C + S_C], bf16, tag="gw")
                    # alternate the two HWDGE issuers (SP + ACT) in steady
                    # state to halve per-ring FIFO pressure; keep early tiles
                    # on SP so the exp chain on ACT isn't stuck behind DMA
                    # slot-waits
                    dma_eng = nc.scalar if (k >= 16 and k % 2 == 1) else nc.sync
                    dma_eng.dma_start(out=gw_sb[:], in_=gw_d[k, :, :])
                for j in range(SUBT):
                    nc.tensor.matmul(
                        acc[j][:],
                        gw_sb[:, B_C + j * P : B_C + (j + 1) * P],
                        gw_sb[:, 0:B_C],
                        start=(k == 0),
                        stop=(k == K_TILES - 1),
                    )

            # --- normalize each output row by 1/denom (per-partition scalar)
            # and store; host transposes at assembly. Split across DVE and ACT
            # (activation Copy with a per-partition scale AP) so the four
            # scales run pairwise-parallel instead of serializing on DVE ---
            for j in range(SUBT):
                o_sb = opool.tile([P, B_C], f32, tag="osb")
                if j % 2 == 0:
                    nc.vector.tensor_scalar_mul(
                        o_sb[:], acc[j][:], recip_sb[:, j : j + 1]
                    )
                else:
                    nc.scalar.activation(
                        o_sb[:],
                        acc[j][:],
                        mybir.ActivationFunctionType.Copy,
                        bias=0.0,
                        scale=recip_sb[:, j : j + 1],
                    )
                nc.sync.dma_start(out=out_d[j * P : (j + 1) * P, :], in_=o_sb[:])

    nc.finalize()
    return nc


def _build_program_raw():
    """Raw-Bass pipeline (final).

    Measured constraints that shaped this design:
      - all DMA queues share a ~285 GB/s aggregate ceiling per core (two
        HWDGE rings at ~130-135 GB/s each; extra software-DGE queues add
        nothing net), so the 16.25 MB input stream is DMA-pinned at ~60us:
        the kernel is input-bandwidth-bound, slightly above the PE's 57.5us.
      - gw input tiles are 256KB singles alternating the SP/ACT rings with
        one completion semaphore per DMA (ring completion is unordered).
      - plog is delayed to the gw stream's tail (gpsimd software queue) so
        it does not steal early bandwidth; exp/recip still complete well
        before the normalize needs them.
      - after a PE drain, the output is normalized into bf16 (DVE j0,j2;
        ACT j1,j3 via Copy-with-scale) and stored as four 128KB partition-
        split DMAs running in parallel on both rings (the host upcasts).
      - 128 dependency-free warmups on the pre-barrier const tile cover
        first-tile DMA latency and hold the PE clock at 8/8.
      - NOTE: normalizing/storing per-PSUM-bank while the PE is still
        streaming matmuls (j-major tail) hangs the device; keep all acc
        reads behind the drain.
    """
    import concourse.bass as bass
    import concourse.mybir as mybir

    f32 = mybir.dt.float32
    bf16 = mybir.dt.bfloat16
    FD = B_C + S_C  # fused free dim per K-tile: 1024
    NBUF = 33
    SUBT = S_C // P  # 4
    WARMUP = 128
    K_TAIL = K_TILES - 4  # j-major region: tiles 60..63
    SP, ACT, DVE = 0, 1, 2

    # input DMA plan: tile 0 split G|W across SP/ACT; the software DGE
    # queue (gpsimd; ~75 GB/s) takes every 6th tile, the rest alternate
    # over the two ~135 GB/s hardware rings -> ~345 GB/s aggregate vs the
    # PE's 300 GB/s demand. (DVE HWDGE is rejected by the NEFF compiler.)
    SW = DVE  # ring id 2 = software DGE on gpsimd
    # v4 measured: the DMA queues share a ~285 GB/s aggregate ceiling, so a
    # third (software) queue adds no net input bandwidth and its slow first
    # tile stalled the PE 3us. gw rides the two fast HWDGE rings only.
    sw_tiles = set()
    tiles = [(0, SP), (0, ACT)]
    hw_ring = 0
    for k in range(1, K_TILES):
        if k in sw_tiles:
            tiles.append((k, SW))
        else:
            tiles.append((k, (SP, ACT)[hw_ring]))
            hw_ring ^= 1

    nc = bass.Bass("TRN2", dynamic_dma_scratch_size=65536)
    gw_d = nc.dram_tensor("gw", [P, K_TILES * FD], bf16, kind="ExternalInput")
    plog_d = nc.dram_tensor("plog", [P, SUBT, PAD_SLOTS], f32, kind="ExternalInput")
    out_d = nc.dram_tensor("out", [P, SUBT, B_C], bf16, kind="ExternalOutput")

    from contextlib import ExitStack

    with ExitStack() as ctx:
        gw_sb = ctx.enter_context(nc.sbuf_tensor([P, NBUF * FD], bf16))
        plog_sb = ctx.enter_context(nc.sbuf_tensor([P, SUBT, PAD_SLOTS], f32))
        exp_sb = ctx.enter_context(nc.sbuf_tensor([P, SUBT, PAD_SLOTS], f32))
        den_sb = ctx.enter_context(nc.sbuf_tensor([P, SUBT], f32))
        recip_sb = ctx.enter_context(nc.sbuf_tensor([P, SUBT], f32))
        o_sb = ctx.enter_context(nc.sbuf_tensor([P, SUBT, B_C], bf16))
        acc_ps = ctx.enter_context(nc.psum_tensor([P, SUBT, B_C], f32))
        scratch_ps = ctx.enter_context(nc.psum_tensor([1, 1], f32))
        s_tile = [
            ctx.enter_context(nc.semaphore(name=f"s_t{i}"))
            for i in range(len(tiles))
        ]
        s_acc = [
            ctx.enter_context(nc.semaphore(name=f"s_acc{j}")) for j in range(SUBT)
        ]
        # j3 is normalized in two halves (DVE cols 0:256, ACT cols 256:512)
        s_n3b = ctx.enter_context(nc.semaphore(name="s_n3b"))
        s_norm = [
            ctx.enter_context(nc.semaphore(name=f"s_norm{j}")) for j in range(SUBT)
        ]
        s_plog = ctx.enter_context(nc.semaphore(name="s_plog"))
        s_exp = ctx.enter_context(nc.semaphore(name="s_exp"))
        s_den = ctx.enter_context(nc.semaphore(name="s_den"))
        s_recip = ctx.enter_context(nc.semaphore(name="s_recip"))
        s_mm = ctx.enter_context(nc.semaphore(name="s_mm"))
        s_fin = ctx.enter_context(nc.semaphore(name="s_fin"))
        s_done = ctx.enter_context(nc.semaphore(name="s_done"))
        block = ctx.enter_context(nc.Block(no_gpsimd_drain=True))

        def emit_gw_dmas(eng, my_ring):
            for ti, (k, r) in enumerate(tiles):
                if r != my_ring:
                    continue
                if k + 1 > NBUF:
                    eng.wait_ge(s_mm, k + 1 - NBUF)
                slot = k % NBUF
                if k == 0:  # half-tile DMAs for tile 0
                    c0, c1 = (0, B_C) if my_ring == SP else (B_C, FD)
                    eng.dma_start(
                        gw_sb[:, slot * FD + c0 : slot * FD + c1],
                        gw_d[:, k * FD + c0 : k * FD + c1],
                    ).then_inc(s_tile[ti], 16)
                else:
                    eng.dma_start(
                        gw_sb[:, slot * FD : (slot + 1) * FD],
                        gw_d[:, k * FD : (k + 1) * FD],
                    ).then_inc(s_tile[ti], 16)

        # k -> tile-sem indices the PE must wait on
        tile_sems = {}
        for ti, (k, r) in enumerate(tiles):
            tile_sems.setdefault(k, []).append(ti)

        @block.sync
        def _(sync):
            emit_gw_dmas(sync, SP)
            sync.wait_ge(s_norm[0], 1)
            sync.wait_ge(s_norm[1], 1)
            sync.dma_start(
                out_d[0:64, 0:2, :], o_sb[0:64, 0:2, :]
            ).then_inc(s_done, 16)
            sync.wait_ge(s_norm[2], 1)
            sync.wait_ge(s_norm[3], 1)
            sync.dma_start(
                out_d[0:64, 2:4, :], o_sb[0:64, 2:4, :]
            ).then_inc(s_done, 16)

        @block.scalar
        def _(scalar):
            emit_gw_dmas(scalar, ACT)
            scalar.wait_ge(s_plog, 16)
            scalar.activation(
                exp_sb[:], plog_sb[:], mybir.ActivationFunctionType.Exp
            ).then_inc(s_exp, 1)
            scalar.wait_ge(s_recip, 1)
            scalar.wait_ge(s_fin, 1)
            for j in (1, 3):
                scalar.activation(
                    o_sb[:, j, :],
                    acc_ps[:, j, :],
                    mybir.ActivationFunctionType.Copy,
                    bias=0.0,
                    scale=recip_sb[:, j : j + 1],
                ).then_inc(s_norm[j], 1)
            scalar.wait_ge(s_norm[0], 1)
            scalar.wait_ge(s_norm[1], 1)
            scalar.dma_start(
                out_d[64:128, 0:2, :], o_sb[64:128, 0:2, :]
            ).then_inc(s_done, 16)
            scalar.wait_ge(s_norm[2], 1)
            scalar.wait_ge(s_norm[3], 1)
            scalar.dma_start(
                out_d[64:128, 2:4, :], o_sb[64:128, 2:4, :]
            ).then_inc(s_done, 16)

        @block.tensor
        def _(tensor):
            # dependency-free warmups on the pre-barrier const tile keep the
            # HAM clock-gate busy so the PE reaches 8/8 before the stream
            const_one = nc.const_aps.aps[(bf16, 1.0)]
            for _ in range(WARMUP):
                tensor.matmul(
                    scratch_ps[:], const_one, const_one, start=True, stop=True
                )
            for k in range(K_TILES):
                for ti in tile_sems[k]:
                    tensor.wait_ge(s_tile[ti], 16)
                slot = k % NBUF
                tile = gw_sb[:, slot * FD : (slot + 1) * FD]
                for j in range(SUBT):
                    mm = tensor.matmul(
                        acc_ps[:, j, :],
                        tile[:, B_C + j * P : B_C + (j + 1) * P],
                        tile[:, 0:B_C],
                        start=(k == 0),
                        stop=(k == K_TILES - 1),
                    )
                    if j == SUBT - 1:
                        # operands fully streamed at retire -> slot reusable
                        mm.then_inc(s_mm, 1)
            # drain flushes the PSUM writeback before DVE/ACT read acc
            tensor.drain().then_inc(s_fin, 1)

        @block.vector
        def _(vector):
            vector.wait_ge(s_exp, 1)
            vector.tensor_reduce(
                out=den_sb[:],
                in_=exp_sb[:],
                op=mybir.AluOpType.add,
                axis=mybir.AxisListType.X,
            ).then_inc(s_den, 1)
            # same-engine RAW still needs a sem edge (DVE pipelines insts)
            vector.wait_ge(s_den, 1)
            vector.reciprocal(recip_sb[:], den_sb[:]).then_inc(s_recip, 1)
            vector.wait_ge(s_recip, 1)
            vector.wait_ge(s_fin, 1)
            for j in (0, 2):
                vector.tensor_scalar_mul(
                    o_sb[:, j, :], acc_ps[:, j, :], recip_sb[:, j : j + 1]
                ).then_inc(s_norm[j], 1)

        @block.gpsimd
        def _(gpsimd):
            # plog rides the shared ~285 GB/s DMA ceiling: delay it past the
            # early gw crunch (recip is only needed at stream end)
            gpsimd.wait_ge(s_mm, 30)
            gpsimd.dma_start(plog_sb[:], plog_d[:, :, :]).then_inc(s_plog, 16)
            gpsimd.wait_ge(s_done, 16 * 4)

    nc.finalize()
    return nc


def _get_program():
    if "nc" not in _PROGRAM_CACHE:
        _PROGRAM_CACHE["nc"] = _build_program_raw()
    return _PROGRAM_CACHE["nc"]


def _ensure_ntff_hook():
    """Make NTFF profiling under axon work (BASS_TRACE=1): the image's antenv
    package lacks the axon_hooks holder module, so synthesize it and register
    the ctypes-based profile hook from trn_agent_boot. Best-effort."""
    import types

    try:
        import antenv

        try:
            from antenv.axon_hooks import get_axon_ntff_profile_hook  # noqa: F401

            return  # already present and registered
        except ImportError:
            pass
        mod = types.ModuleType("antenv.axon_hooks")
        _holder = [None]
        mod.set_axon_ntff_profile_hook = lambda h: _holder.__setitem__(0, h)
        mod.get_axon_ntff_profile_hook = lambda: _holder[0]
        sys.modules["antenv.axon_hooks"] = mod
        antenv.axon_hooks = mod

        from trn_agent_boot.trn_boot import _ntff_profile_via_ctypes

        hook = _ntff_profile_via_ctypes("/opt/axon/libaxon_pjrt.so")
        mod.set_axon_ntff_profile_hook(hook)
    except Exception:
        pass


def kernel(**inputs):
    global LAST_RESULTS
    G = np.asarray(inputs["geneset_features"], dtype=np.float32)
    logits = np.asarray(inputs["attn_logits"], dtype=np.float32)
    flat_idx = np.asarray(inputs["flat_idx"]).astype(np.int64)
    seg = np.asarray(inputs["segment_ids"]).astype(np.int64)
    T = logits.shape[0]

    # Host-side layout prep: scatter exp(logits) into the sparse aggregation
    # matrix (member sets are sampled without replacement, so (idx, seg) pairs
    # are unique within a set and the fancy assignment is collision-free).
    e32 = np.exp(logits)
    W = np.zeros((NUM_GENESETS, NUM_SETS), dtype=ml_dtypes.bfloat16)
    W[flat_idx, seg] = e32.astype(ml_dtypes.bfloat16)

    # Padded per-set logit columns; device computes denominators from these.
    sizes = np.bincount(seg, minlength=NUM_SETS)
    starts = np.concatenate([[0], np.cumsum(sizes)[:-1]])
    pos = np.arange(T) - starts[seg]
    plogT = np.full((PAD_SLOTS, NUM_SETS), NEG_FILL, dtype=np.float32)
    plogT[pos, seg] = logits

    Gb = G.astype(ml_dtypes.bfloat16)

    GbT = np.ascontiguousarray(Gb.T)  # (8192, 1024)
    in_maps = []
    for c in range(N_CORES):
        bg, sg = divmod(c, SG)
        gt = GbT[:, bg * B_C : (bg + 1) * B_C].reshape(K_TILES, P, B_C)
        w = W[:, sg * S_C : (sg + 1) * S_C].reshape(K_TILES, P, S_C)
        # flat per-partition-contiguous layout: gw[p, k*FD + c] so one DMA
        # can move multiple K-tiles as large contiguous descriptor rows
        gw = (
            np.concatenate([gt, w], axis=2)  # (K_TILES, P, FD)
            .transpose(1, 0, 2)
            .reshape(P, K_TILES * (B_C + S_C))
        )
        # sets-on-partitions layout: plog[s_local, j*128+t] = logit slot t
        # of set (sg*S_C + j*128 + s_local)
        chunk = plogT[:, sg * S_C : (sg + 1) * S_C]  # (slots, S_C)
        plog = np.ascontiguousarray(
            chunk.reshape(PAD_SLOTS, S_C // P, P).transpose(2, 1, 0)
        )  # (P, SUBT, PAD_SLOTS)
        in_maps.append({"gw": np.ascontiguousarray(gw), "plog": plog})

    from concourse.bass_utils import run_bass_kernel_spmd

    _ensure_ntff_hook()
    nc = _get_program()
    res = run_bass_kernel_spmd(nc, in_maps, core_ids=list(range(N_CORES)))
    LAST_RESULTS = res

    out = np.empty((BATCH, NUM_SETS), dtype=np.float32)
    for c in range(N_CORES):
        bg, sg = divmod(c, SG)
        # device out is [P, SUBT, B_C] bf16 with set s = j*128 + p
        ot = np.asarray(res.results[c]["out"]).astype(np.float32)
        block = ot.transpose(1, 0, 2).reshape(S_C, B_C)
        out[bg * B_C : (bg + 1) * B_C, sg * S_C : (sg + 1) * S_C] = block.T
    return out



# revision 25
# speedup vs baseline: 1.0190x; 1.0190x over previous
"""Trainium2 Bass kernel for CellPathwayAttentionAggregator (segment-reduce).

Math: out[b, s] = sum_{i in set s} softmax_s(attn_logits)[i] * G[b, flat_idx[i]]

Device decomposition (per core, transposed output):
    out^T = (W_exp^T @ G^T) * (1 / denom)[:, None]
where W_exp[g, s] = sum_{i in set s, flat_idx[i]=g} exp(attn_logits[i]) is the
(unnormalized) sparse aggregation matrix, scattered on the host as pure layout
prep (elementwise exp + scatter; no reductions on host), and
    denom[s] = sum_{i in set s} exp(attn_logits[i])
is computed ON DEVICE from a sets-on-partitions padded logits tile (ACT exp ->
DVE free-axis reduce -> DVE reciprocal; no PE involvement), followed by an
on-device per-partition normalization of the matmul output. The host
transposes each core's (sets x batch) block during assembly.

Sharding: 8 cores = 2 batch groups (512 rows) x 4 set groups (512 sets).
Each core accumulates a (512 x 8192) @ (8192 x 512) bf16 matmul in fp32 PSUM
over 64 K-tiles (4 set-subtile PSUM banks, N=512 moving operand), with a
dependency-free PE warmup against the HAM clock-gate and input tiles streamed
as 256KB G^T|W DMAs alternating the two HWDGE rings (which share a ~285 GB/s
aggregate ceiling - the kernel is input-bandwidth-bound). The output is
normalized into bf16 (host upcasts) and stored as partition-split DMAs on
both rings; the padded-logits load is deferred to the stream's tail.
"""

import sys

if "/opt/trn_rl_repo" not in sys.path:
    sys.path.insert(0, "/opt/trn_rl_repo")

import ml_dtypes
import numpy as np

NUM_SETS = 2048
NUM_GENESETS = 8192
BATCH = 1024
N_CORES = 8
BG, SG = 2, 4  # batch groups x set groups (BG*SG == N_CORES)
B_C = BATCH // BG  # 512 batch rows per core
S_C = NUM_SETS // SG  # 512 sets per core
P = 128
K_TILES = NUM_GENESETS // P  # 64
M_TILES = B_C // P  # 4
PAD_SLOTS = 128  # >= MAX set size (120)
NEG_FILL = -87.0  # exp(-87) ~ 1.6e-38 ~ 0 in fp32

_PROGRAM_CACHE = {}
LAST_RESULTS = None  # BassKernelResults of the most recent run (for profiling)


def _build_program():
    import concourse.mybir as mybir
    from concourse import bacc
    from concourse.tile import TileContext

    f32 = mybir.dt.float32
    bf16 = mybir.dt.bfloat16

    nc = bacc.Bacc("TRN2", target_bir_lowering=False, debug=False)
    # fused per-K-tile input: [:, :, :B_C] = G^T tile, [:, :, B_C:] = W tile.
    # One DMA per K-tile keeps every matmul's sync-wait count at <=1 (the
    # S3 LDWEIGHTS encoding only has a single wait slot).
    gw_d = nc.dram_tensor("gw", [K_TILES, P, B_C + S_C], bf16, kind="ExternalInput")
    plog_d = nc.dram_tensor(
        "plog", [P, (S_C // P) * PAD_SLOTS], f32, kind="ExternalInput"
    )
    out_d = nc.dram_tensor("out", [S_C, B_C], f32, kind="ExternalOutput")

    with TileContext(nc) as tc:
        with (
            tc.tile_pool(name="const", bufs=1) as cpool,
            tc.tile_pool(name="gw", bufs=12) as gwpool,
            tc.tile_pool(name="outp", bufs=4) as opool,
            tc.tile_pool(name="ps", bufs=1, space="PSUM") as ppool,
        ):
            # --- PE warmup: dependency-free N=1 matmuls on the pre-barrier
            # const tile keep the HAM clock-gate busy from right after the
            # entry barrier, so it reaches 8/8 (2.4 GHz) before the real
            # stream starts.
            const_one = nc.const_aps.aps[(bf16, 1.0)]
            scratch_ps = ppool.tile([1, 1], f32, tag="scratch")
            for _ in range(64):
                nc.tensor.matmul(
                    scratch_ps[:], const_one, const_one, start=True, stop=True
                )

            # --- tile 0 split across BOTH HWDGE rings (G-half on SP, W-half
            # on ACT) so the first matmul's data lands ~1us sooner; emitted
            # before the exp so ACT's ring isn't blocked behind the plog wait
            gw0 = gwpool.tile([P, B_C + S_C], bf16, tag="gw", name="gw0")
            nc.sync.dma_start(out=gw0[:, 0:B_C], in_=gw_d[0, :, 0:B_C])
            nc.scalar.dma_start(
                out=gw0[:, B_C : B_C + S_C], in_=gw_d[0, :, B_C : B_C + S_C]
            )

            # --- denominator chain: sets live on the PARTITION axis, so it
            # needs no PE matmuls at all (ACT exp -> DVE free-axis reduce ->
            # DVE reciprocal), fully parallel to the matmul stream ---
            SUBT = S_C // P  # 4 set-subtiles of 128 sets
            plog_sb = cpool.tile([P, SUBT * PAD_SLOTS], f32, tag="plog")
            nc.gpsimd.dma_start(out=plog_sb[:], in_=plog_d[:, :])
            exp_sb = cpool.tile([P, SUBT * PAD_SLOTS], f32, tag="exp")
            nc.scalar.activation(
                exp_sb[:], plog_sb[:], mybir.ActivationFunctionType.Exp
            )
            den_sb = cpool.tile([P, SUBT], f32, tag="den")
            nc.vector.tensor_reduce(
                out=den_sb[:],
                in_=exp_sb[:].rearrange("p (j t) -> p j t", t=PAD_SLOTS),
                op=mybir.AluOpType.add,
                axis=mybir.AxisListType.X,
            )
            recip_sb = cpool.tile([P, SUBT], f32, tag="recip")
            nc.vector.reciprocal(recip_sb[:], den_sb[:])

            # --- main matmul: out^T = W_c^T @ G_c^T, accumulated over 64
            # K-tiles; output has sets on partitions, batch on free ---
            acc = [
                ppool.tile([P, B_C], f32, tag=f"acc{j}", name=f"acc{j}")
                for j in range(SUBT)
            ]
            for k in range(K_TILES):
                if k == 0:
                    gw_sb = gw0
                else:
                    gw_sb = gwpool.tile([P, B_C + S_C], bf16, tag="gw")
                    # alternate the two HWDGE issuers (SP + ACT) in steady
                    # state to halve per-ring FIFO pressure; keep early tiles
                    # on SP so the exp chain on ACT isn't stuck behind DMA
                    # slot-waits
                    dma_eng = nc.scalar if (k >= 16 and k % 2 == 1) else nc.sync
                    dma_eng.dma_start(out=gw_sb[:], in_=gw_d[k, :, :])
                for j in range(SUBT):
                    nc.tensor.matmul(
                        acc[j][:],
                        gw_sb[:, B_C + j * P : B_C + (j + 1) * P],
                        gw_sb[:, 0:B_C],
                        start=(k == 0),
                        stop=(k == K_TILES - 1),
                    )

            # --- normalize each output row by 1/denom (per-partition scalar)
            # and store; host transposes at assembly. Split across DVE and ACT
            # (activation Copy with a per-partition scale AP) so the four
            # scales run pairwise-parallel instead of serializing on DVE ---
            for j in range(SUBT):
                o_sb = opool.tile([P, B_C], f32, tag="osb")
                if j % 2 == 0:
                    nc.vector.tensor_scalar_mul(
                        o_sb[:], acc[j][:], recip_sb[:, j : j + 1]
                    )
                else:
                    nc.scalar.activation(
                        o_sb[:],
                        acc[j][:],
                        mybir.ActivationFunctionType.Copy,
                        bias=0.0,
                        scale=recip_sb[:, j : j + 1],
                    )
                nc.sync.dma_start(out=out_d[j * P : (j + 1) * P, :], in_=o_sb[:])

    nc.finalize()
    return nc


def _build_program_raw():
    """Raw-Bass pipeline (final).

    Measured constraints that shaped this design:
      - all DMA queues share a ~285 GB/s aggregate ceiling per core (two
        HWDGE rings at ~130-135 GB/s each; extra software-DGE queues add
        nothing net), so the 16.25 MB input stream is DMA-pinned at ~60us:
        the kernel is input-bandwidth-bound, slightly above the PE's 57.5us.
      - gw input tiles are 256KB singles alternating the SP/ACT rings with
        one completion semaphore per DMA (ring completion is unordered).
      - plog is delayed to the gw stream's tail (gpsimd software queue) so
        it does not steal early bandwidth; exp/recip still complete well
        before the normalize needs them.
      - after a PE drain, the output is normalized into bf16 (DVE j0,j2;
        ACT j1,j3 via Copy-with-scale) and stored as four 128KB partition-
        split DMAs running in parallel on both rings (the host upcasts).
      - 128 dependency-free warmups on the pre-barrier const tile cover
        first-tile DMA latency and hold the PE clock at 8/8.
      - NOTE: normalizing/storing per-PSUM-bank while the PE is still
        streaming matmuls (j-major tail) hangs the device; keep all acc
        reads behind the drain.
    """
    import concourse.bass as bass
    import concourse.mybir as mybir

    f32 = mybir.dt.float32
    bf16 = mybir.dt.bfloat16
    FD = B_C + S_C  # fused free dim per K-tile: 1024
    NBUF = 33
    SUBT = S_C // P  # 4
    WARMUP = 128
    K_TAIL = K_TILES - 4  # j-major region: tiles 60..63
    SP, ACT, DVE = 0, 1, 2

    # input DMA plan: tile 0 split G|W across SP/ACT; the software DGE
    # queue (gpsimd; ~75 GB/s) takes every 6th tile, the rest alternate
    # over the two ~135 GB/s hardware rings -> ~345 GB/s aggregate vs the
    # PE's 300 GB/s demand. (DVE HWDGE is rejected by the NEFF compiler.)
    SW = DVE  # ring id 2 = software DGE on gpsimd
    # v4 measured: the DMA queues share a ~285 GB/s aggregate ceiling, so a
    # third (software) queue adds no net input bandwidth and its slow first
    # tile stalled the PE 3us. gw rides the two fast HWDGE rings only.
    sw_tiles = set()
    tiles = [(0, SP), (0, ACT)]
    hw_ring = 0
    for k in range(1, K_TILES):
        if k in sw_tiles:
            tiles.append((k, SW))
        else:
            tiles.append((k, (SP, ACT)[hw_ring]))
            hw_ring ^= 1

    nc = bass.Bass("TRN2", dynamic_dma_scratch_size=65536)
    gw_d = nc.dram_tensor("gw", [P, K_TILES * FD], bf16, kind="ExternalInput")
    # bf16 padded logits halve plog's share of the saturated DMA ceiling;
    # exp upconverts to fp32 on the ACT path (denominator err ~0.5%)
    plog_d = nc.dram_tensor("plog", [P, SUBT, PAD_SLOTS], bf16, kind="ExternalInput")
    out_d = nc.dram_tensor("out", [P, SUBT, B_C], bf16, kind="ExternalOutput")

    from contextlib import ExitStack

    with ExitStack() as ctx:
        gw_sb = ctx.enter_context(nc.sbuf_tensor([P, NBUF * FD], bf16))
        plog_sb = ctx.enter_context(nc.sbuf_tensor([P, SUBT, PAD_SLOTS], bf16))
        exp_sb = ctx.enter_context(nc.sbuf_tensor([P, SUBT, PAD_SLOTS], f32))
        den_sb = ctx.enter_context(nc.sbuf_tensor([P, SUBT], f32))
        recip_sb = ctx.enter_context(nc.sbuf_tensor([P, SUBT], f32))
        o_sb = ctx.enter_context(nc.sbuf_tensor([P, SUBT, B_C], bf16))
        acc_ps = ctx.enter_context(nc.psum_tensor([P, SUBT, B_C], f32))
        scratch_ps = ctx.enter_context(nc.psum_tensor([1, 1], f32))
        s_tile = [
            ctx.enter_context(nc.semaphore(name=f"s_t{i}"))
            for i in range(len(tiles))
        ]
        s_acc = [
            ctx.enter_context(nc.semaphore(name=f"s_acc{j}")) for j in range(SUBT)
        ]
        # j3 is normalized in two halves (DVE cols 0:256, ACT cols 256:512)
        s_n3b = ctx.enter_context(nc.semaphore(name="s_n3b"))
        s_norm = [
            ctx.enter_context(nc.semaphore(name=f"s_norm{j}")) for j in range(SUBT)
        ]
        s_plog = ctx.enter_context(nc.semaphore(name="s_plog"))
        s_exp = ctx.enter_context(nc.semaphore(name="s_exp"))
        s_den = ctx.enter_context(nc.semaphore(name="s_den"))
        s_recip = ctx.enter_context(nc.semaphore(name="s_recip"))
        s_mm = ctx.enter_context(nc.semaphore(name="s_mm"))
        s_fin = ctx.enter_context(nc.semaphore(name="s_fin"))
        s_done = ctx.enter_context(nc.semaphore(name="s_done"))
        block = ctx.enter_context(nc.Block(no_gpsimd_drain=True))

        def emit_gw_dmas(eng, my_ring):
            for ti, (k, r) in enumerate(tiles):
                if r != my_ring:
                    continue
                if k + 1 > NBUF:
                    eng.wait_ge(s_mm, k + 1 - NBUF)
                slot = k % NBUF
                if k == 0:  # half-tile DMAs for tile 0
                    c0, c1 = (0, B_C) if my_ring == SP else (B_C, FD)
                    eng.dma_start(
                        gw_sb[:, slot * FD + c0 : slot * FD + c1],
                        gw_d[:, k * FD + c0 : k * FD + c1],
                    ).then_inc(s_tile[ti], 16)
                else:
                    eng.dma_start(
                        gw_sb[:, slot * FD : (slot + 1) * FD],
                        gw_d[:, k * FD : (k + 1) * FD],
                    ).then_inc(s_tile[ti], 16)

        # k -> tile-sem indices the PE must wait on
        tile_sems = {}
        for ti, (k, r) in enumerate(tiles):
            tile_sems.setdefault(k, []).append(ti)

        @block.sync
        def _(sync):
            emit_gw_dmas(sync, SP)
            sync.wait_ge(s_norm[0], 1)
            sync.wait_ge(s_norm[1], 1)
            sync.dma_start(
                out_d[0:64, 0:2, :], o_sb[0:64, 0:2, :]
            ).then_inc(s_done, 16)
            sync.wait_ge(s_norm[2], 1)
            sync.wait_ge(s_norm[3], 1)
            sync.dma_start(
                out_d[0:64, 2:4, :], o_sb[0:64, 2:4, :]
            ).then_inc(s_done, 16)

        @block.scalar
        def _(scalar):
            emit_gw_dmas(scalar, ACT)
            scalar.wait_ge(s_plog, 16)
            scalar.activation(
                exp_sb[:], plog_sb[:], mybir.ActivationFunctionType.Exp
            ).then_inc(s_exp, 1)
            scalar.wait_ge(s_recip, 1)
            scalar.wait_ge(s_fin, 1)
            for j in (1, 3):
                scalar.activation(
                    o_sb[:, j, :],
                    acc_ps[:, j, :],
                    mybir.ActivationFunctionType.Copy,
                    bias=0.0,
                    scale=recip_sb[:, j : j + 1],
                ).then_inc(s_norm[j], 1)
            scalar.wait_ge(s_norm[0], 1)
            scalar.wait_ge(s_norm[1], 1)
            scalar.dma_start(
                out_d[64:128, 0:2, :], o_sb[64:128, 0:2, :]
            ).then_inc(s_done, 16)
            scalar.wait_ge(s_norm[2], 1)
            scalar.wait_ge(s_norm[3], 1)
            scalar.dma_start(
                out_d[64:128, 2:4, :], o_sb[64:128, 2:4, :]
            ).then_inc(s_done, 16)

        @block.tensor
        def _(tensor):
            # dependency-free warmups on the pre-barrier const tile keep the
            # HAM clock-gate busy so the PE reaches 8/8 before the stream
            const_one = nc.const_aps.aps[(bf16, 1.0)]
            for _ in range(WARMUP):
                tensor.matmul(
                    scratch_ps[:], const_one, const_one, start=True, stop=True
                )
            for k in range(K_TILES):
                for ti in tile_sems[k]:
                    tensor.wait_ge(s_tile[ti], 16)
                slot = k % NBUF
                tile = gw_sb[:, slot * FD : (slot + 1) * FD]
                for j in range(SUBT):
                    mm = tensor.matmul(
                        acc_ps[:, j, :],
                        tile[:, B_C + j * P : B_C + (j + 1) * P],
                        tile[:, 0:B_C],
                        start=(k == 0),
                        stop=(k == K_TILES - 1),
                    )
                    if j == SUBT - 1:
                        # operands fully streamed at retire -> slot reusable
                        mm.then_inc(s_mm, 1)
            # drain flushes the PSUM writeback before DVE/ACT read acc
            tensor.drain().then_inc(s_fin, 1)

        @block.vector
        def _(vector):
            vector.wait_ge(s_exp, 1)
            vector.tensor_reduce(
                out=den_sb[:],
                in_=exp_sb[:],
                op=mybir.AluOpType.add,
                axis=mybir.AxisListType.X,
            ).then_inc(s_den, 1)
            # same-engine RAW still needs a sem edge (DVE pipelines insts)
            vector.wait_ge(s_den, 1)
            vector.reciprocal(recip_sb[:], den_sb[:]).then_inc(s_recip, 1)
            vector.wait_ge(s_recip, 1)
            vector.wait_ge(s_fin, 1)
            for j in (0, 2):
                vector.tensor_scalar_mul(
                    o_sb[:, j, :], acc_ps[:, j, :], recip_sb[:, j : j + 1]
                ).then_inc(s_norm[j], 1)

        @block.gpsimd
        def _(gpsimd):
            # plog rides the shared ~285 GB/s DMA ceiling: delay it past the
            # early gw crunch (recip is only needed at stream end)
            gpsimd.wait_ge(s_mm, 30)
            gpsimd.dma_start(plog_sb[:], plog_d[:, :, :]).then_inc(s_plog, 16)
            gpsimd.wait_ge(s_done, 16 * 4)

    nc.finalize()
    return nc


def _get_program():
    if "nc" not in _PROGRAM_CACHE:
        _PROGRAM_CACHE["nc"] = _build_program_raw()
    return _PROGRAM_CACHE["nc"]


def _ensure_ntff_hook():
    """Make NTFF profiling under axon work (BASS_TRACE=1): the image's antenv
    package lacks the axon_hooks holder module, so synthesize it and register
    the ctypes-based profile hook from trn_agent_boot. Best-effort."""
    import types

    try:
        import antenv

        try:
            from antenv.axon_hooks import get_axon_ntff_profile_hook  # noqa: F401

            return  # already present and registered
        except ImportError:
            pass
        mod = types.ModuleType("antenv.axon_hooks")
        _holder = [None]
        mod.set_axon_ntff_profile_hook = lambda h: _holder.__setitem__(0, h)
        mod.get_axon_ntff_profile_hook = lambda: _holder[0]
        sys.modules["antenv.axon_hooks"] = mod
        antenv.axon_hooks = mod

        from trn_agent_boot.trn_boot import _ntff_profile_via_ctypes

        hook = _ntff_profile_via_ctypes("/opt/axon/libaxon_pjrt.so")
        mod.set_axon_ntff_profile_hook(hook)
    except Exception:
        pass


def kernel(**inputs):
    global LAST_RESULTS
    G = np.asarray(inputs["geneset_features"], dtype=np.float32)
    logits = np.asarray(inputs["attn_logits"], dtype=np.float32)
    flat_idx = np.asarray(inputs["flat_idx"]).astype(np.int64)
    seg = np.asarray(inputs["segment_ids"]).astype(np.int64)
    T = logits.shape[0]

    # Host-side layout prep: scatter exp(logits) into the sparse aggregation
    # matrix (member sets are sampled without replacement, so (idx, seg) pairs
    # are unique within a set and the fancy assignment is collision-free).
    e32 = np.exp(logits)
    W = np.zeros((NUM_GENESETS, NUM_SETS), dtype=ml_dtypes.bfloat16)
    W[flat_idx, seg] = e32.astype(ml_dtypes.bfloat16)

    # Padded per-set logit columns; device computes denominators from these.
    sizes = np.bincount(seg, minlength=NUM_SETS)
    starts = np.concatenate([[0], np.cumsum(sizes)[:-1]])
    pos = np.arange(T) - starts[seg]
    plogT = np.full((PAD_SLOTS, NUM_SETS), NEG_FILL, dtype=np.float32)
    plogT[pos, seg] = logits

    Gb = G.astype(ml_dtypes.bfloat16)

    GbT = np.ascontiguousarray(Gb.T)  # (8192, 1024)
    in_maps = []
    for c in range(N_CORES):
        bg, sg = divmod(c, SG)
        gt = GbT[:, bg * B_C : (bg + 1) * B_C].reshape(K_TILES, P, B_C)
        w = W[:, sg * S_C : (sg + 1) * S_C].reshape(K_TILES, P, S_C)
        # flat per-partition-contiguous layout: gw[p, k*FD + c] so one DMA
        # can move multiple K-tiles as large contiguous descriptor rows
        gw = (
            np.concatenate([gt, w], axis=2)  # (K_TILES, P, FD)
            .transpose(1, 0, 2)
            .reshape(P, K_TILES * (B_C + S_C))
        )
        # sets-on-partitions layout: plog[s_local, j*128+t] = logit slot t
        # of set (sg*S_C + j*128 + s_local)
        chunk = plogT[:, sg * S_C : (sg + 1) * S_C]  # (slots, S_C)
        plog = np.ascontiguousarray(
            chunk.reshape(PAD_SLOTS, S_C // P, P).transpose(2, 1, 0)
        ).astype(ml_dtypes.bfloat16)  # (P, SUBT, PAD_SLOTS)
        in_maps.append({"gw": np.ascontiguousarray(gw), "plog": plog})

    from concourse.bass_utils import run_bass_kernel_spmd

    _ensure_ntff_hook()
    nc = _get_program()
    res = run_bass_kernel_spmd(nc, in_maps, core_ids=list(range(N_CORES)))
    LAST_RESULTS = res

    out = np.empty((BATCH, NUM_SETS), dtype=np.float32)
    for c in range(N_CORES):
        bg, sg = divmod(c, SG)
        # device out is [P, SUBT, B_C] bf16 with set s = j*128 + p
        ot = np.asarray(res.results[c]["out"]).astype(np.float32)
        block = ot.transpose(1, 0, 2).reshape(S_C, B_C)
        out[bg * B_C : (bg + 1) * B_C, sg * S_C : (sg + 1) * S_C] = block.T
    return out

